# revision 12
# baseline (speedup 1.0000x reference)
"""Trainium2 Bass kernel for CopyMechModule (pointer-generator copy mechanism).

Computes, for inputs of shape B=8, T=512, S=512, H=768, VOCAB=32110:
  p_gen  = sigmoid(concat(dec, seq) @ w_pgen + b_pgen)          [B, T, 1]
  logits[b, t, v] = sum_{s: ids[b,s]==v} attn[b, t, s]          [B, T, VOCAB]

Sharding: pure data parallel, batch b -> core b (8 cores).

Per-core device algorithm (all fp32, exact):
  1. Transpose attn -> attnT [S, T] on PE (matmul against identity).
  2. Build one-hot "dedup + bucket-sort" matrix P on device from the ids
     (P[s, j] = ids[s] == u_pad[j], where u_pad lays out the sorted unique
     ids grouped by 512-wide vocab tile, each group padded to KPAD slots,
     3 groups per 128-partition tile at bases {0, 32, 64}).
     CS = P.T @ attnT gives the segment-summed attn columns [j, t].
  3. For each of the 63 vocab tiles: a single K=KPAD matmul
     CS_group.T @ QQ_group materializes the full [128t, 512v] output tile in
     PSUM (zeros included, duplicate ids already accumulated); QQ is the
     device-built one-hot of (u - vtile_base) vs an uploaded iota row.
  4. Copy PSUM tiles into wide SBUF strips and stream them out with large
     contiguous DMAs (~4 MB each) -> memory-roofline bound HBM writes.

The vocab dim is padded on device to 63*512 = 32256; the host slices to 32110.
"""

import numpy as np
from contextlib import ExitStack

import concourse.bass as bass
import concourse.tile as tile
from concourse import bacc, mybir
from concourse.bass_utils import run_bass_kernel_spmd

B, T, S, H = 8, 512, 512, 768
VOCAB = 32110
VT_W = 512
NVT = (VOCAB + VT_W - 1) // VT_W  # 63
VPAD = NVT * VT_W  # 32256
NCORES = 8
P = 128
GPT = 3  # groups (vocab tiles) per 128-partition slot tile: bases {0, 32, 64}

f32 = mybir.dt.float32

_BUILD_CACHE: dict = {}


def _build(kpad: int):
    """Build + compile the SPMD single-core program (shared by all 8 cores)."""
    assert kpad == 32, f"unsupported kpad {kpad}"
    n_mt = (NVT + GPT - 1) // GPT  # 21 slot tiles of 128 partitions
    JP = n_mt * P  # 2688 slots
    strip_v = 16
    strip_plan = []
    sv = 0
    while sv < NVT:
        nv = min(strip_v, NVT - sv)
        strip_plan.append((sv, nv))
        sv += nv

    nc = bacc.Bacc("TRN2", target_bir_lowering=False, debug=False, num_devices=NCORES)

    attn_d = nc.dram_tensor("attn", [T, S], f32, kind="ExternalInput")
    dec_d = nc.dram_tensor("dec", [T, H], f32, kind="ExternalInput")
    seq_d = nc.dram_tensor("seq", [T, H], f32, kind="ExternalInput")
    w12_d = nc.dram_tensor("w12", [2, H], f32, kind="ExternalInput")
    b_d = nc.dram_tensor("bsc", [1, 1], f32, kind="ExternalInput")
    ids_d = nc.dram_tensor("idsf", [S], f32, kind="ExternalInput")
    u_d = nc.dram_tensor("upad", [JP], f32, kind="ExternalInput")
    ush_d = nc.dram_tensor("ushift", [JP], f32, kind="ExternalInput")
    iota_d = nc.dram_tensor("iota512", [1, VT_W], f32, kind="ExternalInput")
    ident_d = nc.dram_tensor("ident", [P, P], f32, kind="ExternalInput")

    pgen_d = nc.dram_tensor("pgen", [T, 1], f32, kind="ExternalOutput")
    logits_d = nc.dram_tensor("logits", [T, VOCAB], f32, kind="ExternalOutput")

    eq = mybir.AluOpType.is_equal
    mult = mybir.AluOpType.mult
    addop = mybir.AluOpType.add

    with tile.TileContext(nc) as tc, ExitStack() as ctx:
        const = ctx.enter_context(tc.tile_pool(name="const", bufs=1))
        work = ctx.enter_context(tc.tile_pool(name="work", bufs=6))
        tpool = ctx.enter_context(tc.tile_pool(name="tpool", bufs=3))
        rpool = ctx.enter_context(tc.tile_pool(name="rpool", bufs=10))
        aTp = ctx.enter_context(tc.tile_pool(name="aTp", bufs=1))
        csp = ctx.enter_context(tc.tile_pool(name="csp", bufs=1))
        qqp = ctx.enter_context(tc.tile_pool(name="qqp", bufs=1))
        strips = ctx.enter_context(tc.tile_pool(name="strips", bufs=2))
        psum = ctx.enter_context(tc.tile_pool(name="psum", bufs=8, space="PSUM"))

        ident = const.tile([P, P], f32)
        nc.sync.dma_start(ident[:], ident_d[:, :])
        iota_f = const.tile([P, VT_W], f32)
        nc.gpsimd.dma_start(out=iota_f[:], in_=iota_d[0:1, :].to_broadcast([P, VT_W]))
        u_bc = const.tile([P, JP], f32)
        nc.gpsimd.dma_start(out=u_bc[:], in_=u_d[None, :].to_broadcast([P, JP]))

        ids_tiles = []
        for k in range(4):
            idk = const.tile([P, 1], f32, name=f"ids{k}")
            nc.sync.dma_start(idk[:], ids_d[P * k : P * (k + 1), None])
            ids_tiles.append(idk)

        # ---- load attn and transpose it on PE (regular matmul vs identity) ----
        a_tiles = []
        for k in range(4):
            a_k = work.tile([P, S], f32, name=f"a{k}", tag="work")
            nc.sync.dma_start(a_k[:], attn_d[P * k : P * (k + 1), :])
            a_tiles.append(a_k)
        aT_tiles = []
        for l in range(4):
            aT_l = aTp.tile([P, T], f32, name=f"aT{l}")
            for k in range(4):
                pt = psum.tile([P, VT_W], f32, name="pt", tag="ps")[:, :P]
                # out[s, t] = sum_t' attn[t', 128l+s] * I[t', t] = attn.T block
                nc.tensor.matmul(
                    pt[:],
                    lhsT=a_tiles[k][:, P * l : P * (l + 1)],
                    rhs=ident[:],
                    start=True,
                    stop=True,
                )
                nc.vector.tensor_copy(aT_l[:, P * k : P * (k + 1)], pt[:])
            aT_tiles.append(aT_l)

        # ---- CS = P.T @ attnT (dedup + bucket-sort segment sums), QQ one-hots ----
        cs_all = csp.tile([P, n_mt * T], f32)
        qq_all = qqp.tile([P, n_mt * VT_W], f32)
        for mt in range(n_mt):
            pp = psum.tile([P, T], f32, name="pp", tag="ps")
            for k in range(4):
                p_t = tpool.tile([P, P], f32, name="p_t", tag="ptile")
                nc.vector.tensor_scalar(
                    p_t[:],
                    u_bc[:, P * mt : P * (mt + 1)],
                    ids_tiles[k][:, 0:1],
                    None,
                    op0=eq,
                )
                nc.tensor.matmul(
                    pp[:], lhsT=p_t[:], rhs=aT_tiles[k][:],
                    start=(k == 0), stop=(k == 3),
                )
            nc.vector.tensor_copy(cs_all[:, mt * T : (mt + 1) * T], pp[:])

            ush_t = rpool.tile([P, 1], f32, name="ush_t", tag="ush")
            nc.sync.dma_start(ush_t[:], ush_d[P * mt : P * (mt + 1), None])
            nc.vector.tensor_scalar(
                qq_all[:, mt * VT_W : (mt + 1) * VT_W],
                iota_f[:],
                ush_t[:, 0:1],
                None,
                op0=eq,
            )

        # ---- scatter matmuls + strip writes ----
        for rt in range(4):
            for sv, nv in strip_plan:
                strip = strips.tile([P, strip_v * VT_W], f32, name="strip", tag="strip")
                for i in range(nv):
                    vt = sv + i
                    mt, g = divmod(vt, GPT)
                    b0 = g * kpad
                    vw = min(VT_W, VOCAB - vt * VT_W)  # last vtile is 366 wide
                    ps2 = psum.tile([P, VT_W], f32, name="ps2", tag="ps")
                    nc.tensor.matmul(
                        ps2[:, :vw],
                        lhsT=cs_all[b0 : b0 + kpad, mt * T + P * rt : mt * T + P * rt + P],
                        rhs=qq_all[b0 : b0 + kpad, mt * VT_W : mt * VT_W + vw],
                        start=True,
                        stop=True,
                    )
                    nc.vector.tensor_copy(strip[:, i * VT_W : i * VT_W + vw], ps2[:, :vw])
                dma_eng = nc.sync if (rt + sv) % 2 == 0 else nc.scalar
                sw = min((sv + nv) * VT_W, VOCAB) - sv * VT_W
                dma_eng.dma_start(
                    logits_d[P * rt : P * (rt + 1), sv * VT_W : sv * VT_W + sw],
                    strip[:, :sw],
                )

        # ---- p_gen head ----
        w1b = const.tile([P, H], f32)
        nc.gpsimd.dma_start(out=w1b[:], in_=w12_d[0:1, :].to_broadcast([P, H]))
        w2b = const.tile([P, H], f32)
        nc.gpsimd.dma_start(out=w2b[:], in_=w12_d[1:2, :].to_broadcast([P, H]))
        b_bc = const.tile([P, 1], f32)
        nc.gpsimd.dma_start(out=b_bc[:], in_=b_d[0:1, 0:1].to_broadcast([P, 1]))

        for rt in range(4):
            d_t = work.tile([P, H], f32, name="d_t", tag="work")
            nc.sync.dma_start(d_t[:], dec_d[P * rt : P * (rt + 1), :])
            s_t = work.tile([P, H], f32, name="s_t", tag="work")
            nc.sync.dma_start(s_t[:], seq_d[P * rt : P * (rt + 1), :])
            prod1 = work.tile([P, H], f32, name="prod1", tag="work")
            nc.vector.tensor_tensor(out=prod1[:], in0=d_t[:], in1=w1b[:], op=mult)
            r1 = rpool.tile([P, 1], f32, name="r1", tag="rr")
            nc.vector.tensor_reduce(r1[:], prod1[:], axis=mybir.AxisListType.X, op=addop)
            prod2 = work.tile([P, H], f32, name="prod2", tag="work")
            nc.vector.tensor_tensor(out=prod2[:], in0=s_t[:], in1=w2b[:], op=mult)
            r2 = rpool.tile([P, 1], f32, name="r2", tag="rr")
            nc.vector.tensor_reduce(r2[:], prod2[:], axis=mybir.AxisListType.X, op=addop)
            rs = rpool.tile([P, 1], f32, name="rs", tag="rr")
            nc.vector.tensor_add(rs[:], r1[:], r2[:])
            rsb = rpool.tile([P, 1], f32, name="rsb", tag="rr")
            nc.vector.tensor_add(rsb[:], rs[:], b_bc[:])
            pg = rpool.tile([P, 1], f32, name="pg", tag="rr")
            nc.scalar.activation(
                pg[:], rsb[:], mybir.ActivationFunctionType.Sigmoid,
            )
            nc.sync.dma_start(pgen_d[P * rt : P * (rt + 1), :], pg[:])

    nc.compile()
    return nc


def _host_prep(ids_b: np.ndarray, kpad: int):
    """Sorted-unique ids laid out per 512-wide vocab tile; GPT groups of kpad
    slots per 128-partition tile (bases 0/32/64)."""
    u = np.unique(ids_b.astype(np.int64))
    vt_of = u // VT_W
    starts = np.searchsorted(vt_of, vt_of, side="left")
    occ = np.arange(len(u)) - starts
    if len(occ) and occ.max() >= kpad:
        return None, None
    n_mt = (NVT + GPT - 1) // GPT
    JP = n_mt * P
    u_pad = np.full(JP, -1.0, np.float32)
    ushift = np.full(JP, -1.0, np.float32)
    slots = (vt_of // GPT) * P + (vt_of % GPT) * kpad + occ
    u_pad[slots] = u.astype(np.float32)
    ushift[slots] = (u - vt_of * VT_W).astype(np.float32)
    return u_pad, ushift


def kernel(
    decoder_input_embeds,
    sequence_output,
    cross_attentions,
    input_ids_to_copy,
    w_pgen,
    b_pgen,
):
    dec = np.ascontiguousarray(np.asarray(decoder_input_embeds, dtype=np.float32))
    seq = np.ascontiguousarray(np.asarray(sequence_output, dtype=np.float32))
    attn = np.ascontiguousarray(np.asarray(cross_attentions, dtype=np.float32))
    ids = np.asarray(input_ids_to_copy).astype(np.int64)
    w = np.asarray(w_pgen, dtype=np.float32).reshape(2 * H)
    bsc = np.asarray(b_pgen, dtype=np.float32).reshape(1, 1)
    assert dec.shape == (B, T, H) and attn.shape == (B, T, S) and ids.shape == (B, S)

    kpad = 32
    preps = [_host_prep(ids[b], kpad) for b in range(B)]
    assert all(p[0] is not None for p in preps), "vocab-tile group overflow (>32 uniques)"

    if kpad not in _BUILD_CACHE:
        _BUILD_CACHE[kpad] = _build(kpad)
    nc = _BUILD_CACHE[kpad]

    w12 = np.stack([w[:H], w[H:]]).astype(np.float32)
    iota512 = np.arange(VT_W, dtype=np.float32).reshape(1, VT_W)
    ident = np.eye(P, dtype=np.float32)
    in_maps = []
    for b in range(B):
        u_pad, ushift = preps[b]
        in_maps.append(
            {
                "attn": attn[b],
                "dec": dec[b],
                "seq": seq[b],
                "w12": w12,
                "bsc": bsc,
                "idsf": ids[b].astype(np.float32),
                "upad": u_pad,
                "ushift": ushift,
                "iota512": iota512,
                "ident": ident,
            }
        )

    res = run_bass_kernel_spmd(nc, in_maps, core_ids=list(range(NCORES)))

    p_gen = np.stack([res.results[c]["pgen"] for c in range(NCORES)])
    logits = np.stack(
        [res.results[c]["logits"] for c in range(NCORES)]
    )
    return p_gen, logits
